# revision 1
# baseline (speedup 1.0000x reference)
"""Trainium2 Bass kernel for 16-head attention (B=4, S=2048, D=1024).

Sharding: 8 cores = 4 batches x 2 head-groups. Core c handles batch c//2,
heads (c%2)*8 .. +8. Each core computes a partial projection output
[S, D]; the host sums the two head-group partials per batch and adds
b_proj. No collectives.

Per-core layout trick: host feeds x[b] transposed (xT [D, S]), so the QKV
matmuls produce Q^T / K^T in [qkv-col, seq] layout directly, scores are
computed transposed ([sk, sq]) and softmax is done without max-subtraction
(inputs are bounded; exp stays well inside fp32/bf16 range). V is
ones-augmented so the attn@V matmul also yields softmax row-sums for free;
normalization uses a DVE reciprocal + a K=1 outer-product matmul to
broadcast the per-column scale across partitions. The normalize chain for
iteration i is emitted after iteration i+1's matmuls so the PE never waits
on the reciprocal. One unified PSUM pool lets QKV / attention / projection
matmuls interleave freely.
"""

import sys
import os

sys.path.insert(0, "/opt/trn_rl_repo")

import numpy as np
import ml_dtypes

BF = ml_dtypes.bfloat16

DIM = 1024
N_HEADS = 16
HD = 64
B = 4
S = 2048
HPC = 8          # heads per core
GC = HPC * HD    # 512 columns per head-group
N_CORES = 8
SCALE = HD ** -0.5

_CACHE = {}


def _build_bass():
    import concourse.bass as bass
    import concourse.mybir as mybir
    import concourse.tile as tile
    from concourse import bacc

    f32 = mybir.dt.float32
    bf16 = mybir.dt.bfloat16
    EXP = mybir.ActivationFunctionType.Exp

    nc = bacc.Bacc("TRN2", target_bir_lowering=False, debug=False,
                   num_devices=N_CORES)

    xT = nc.dram_tensor("xT", [DIM, S], bf16, kind="ExternalInput").ap()
    wq = nc.dram_tensor("wq", [DIM, GC], bf16, kind="ExternalInput").ap()
    wk = nc.dram_tensor("wk", [DIM, GC], bf16, kind="ExternalInput").ap()
    wv = nc.dram_tensor("wv", [DIM, GC], bf16, kind="ExternalInput").ap()
    wp = nc.dram_tensor("wp", [GC, DIM], bf16, kind="ExternalInput").ap()
    # q/k biases pre-broadcast on host: [128, m-tile*1024], each m block
    # holds the per-partition bias value replicated over 2x512 columns
    bq = nc.dram_tensor("bq", [128, 4096], f32, kind="ExternalInput").ap()
    bk = nc.dram_tensor("bk", [128, 4096], f32, kind="ExternalInput").ap()
    bvb = nc.dram_tensor("bvb", [128, GC], f32, kind="ExternalInput").ap()
    out = nc.dram_tensor("out", [S, DIM], f32, kind="ExternalOutput").ap()

    KD = DIM // 128   # 8 k-tiles over D
    NQ = GC // 128    # 4 tiles over the 512 head-group columns
    NS = S // 512     # 4 seq chunks of 512
    ST = S // 128     # 16 seq tiles of 128

    with tile.TileContext(nc) as tc:
        with tc.tile_pool(name="const", bufs=1) as cp:
            # interleave input DMAs so the first matmuls' operands land first
            xTs, wqs, wks, wvs = [], [], [], []
            for k in range(KD):
                for lst, src, nm in ((xTs, xT, "x"), (wqs, wq, "q"),
                                     (wks, wk, "k"), (wvs, wv, "v")):
                    w = S if nm == "x" else GC
                    t = cp.tile([128, w], bf16, name=f"w{nm}s{k}")
                    nc.sync.dma_start(t[:], src[k * 128:(k + 1) * 128, :])
                    lst.append(t)
            wps = []
            for k in range(NQ):
                t = cp.tile([128, DIM], bf16, name=f"wps{k}")
                nc.sync.dma_start(t[:], wp[k * 128:(k + 1) * 128, :])
                wps.append(t)
            bq_sb = cp.tile([128, 4096], f32, name="bq_sb")
            nc.sync.dma_start(bq_sb[:], bq[:, :])
            bk_sb = cp.tile([128, 4096], f32, name="bk_sb")
            nc.sync.dma_start(bk_sb[:], bk[:, :])
            bvb_sb = cp.tile([128, GC], f32, name="bvb_sb")
            nc.sync.dma_start(bvb_sb[:], bvb[:, :])
            ones_sb = cp.tile([128, 64], bf16, name="ones_sb")
            nc.any.memset(ones_sb[:], 1.0)

            QT = [cp.tile([128, S], bf16, name=f"QT{m}") for m in range(NQ)]
            KT = [cp.tile([128, S], bf16, name=f"KT{m}") for m in range(NQ)]
            # V tiles: per head 65 cols (64 data + trailing ones column)
            Vt = [cp.tile([128, HPC * 65], bf16, name=f"Vt{s}")
                  for s in range(ST)]
            OT = [cp.tile([128, S], bf16, name=f"OT{m}") for m in range(NQ)]

            for s in range(ST):
                ones_cols = Vt[s][:, :].rearrange(
                    "p (h c) -> p h c", c=65)[:, :, 64:65]
                nc.any.memset(ones_cols, 1.0)

            # one psum pool for the whole kernel: tag "s" = 3x 2-bank slots
            # (scores / qkv / proj / pb), tag "o" = 2x 1-bank accumulators
            with tc.tile_pool(name="ps", bufs=1, space="PSUM") as psp, \
                 tc.tile_pool(name="pbuf", bufs=6) as pbufp, \
                 tc.tile_pool(name="un", bufs=4) as unp, \
                 tc.tile_pool(name="rr", bufs=4) as rrp, \
                 tc.tile_pool(name="stg", bufs=3) as stgp:

                def ps_s(name):
                    return psp.tile([128, 1024], f32, tag="s", bufs=3,
                                    name=name)

                # ---- QKV projections ----
                for dst, ws, bias in ((QT, wqs, bq_sb), (KT, wks, bk_sb)):
                    for m in range(NQ):
                        for n2 in range(NS // 2):
                            ps = ps_s(f"qk{m}{n2}{id(ws) % 97}")
                            for k in range(KD):
                                for h in range(2):
                                    nc.tensor.matmul(
                                        ps[:, h * 512:(h + 1) * 512],
                                        lhsT=ws[k][:, m * 128:(m + 1) * 128],
                                        rhs=xTs[k][:, (n2 * 2 + h) * 512:
                                                   (n2 * 2 + h + 1) * 512],
                                        start=(k == 0), stop=(k == KD - 1))
                            nc.vector.tensor_add(
                                dst[m][:, n2 * 1024:(n2 + 1) * 1024], ps[:],
                                bias[:, m * 1024:(m + 1) * 1024])
                for s2 in range(ST // 2):
                    ps = ps_s(f"v{s2}")
                    for k in range(KD):
                        for h in range(2):
                            st = (s2 * 2 + h) * 128
                            nc.tensor.matmul(
                                ps[:, h * 512:(h + 1) * 512],
                                lhsT=xTs[k][:, st:st + 128],
                                rhs=wvs[k][:, :],
                                start=(k == 0), stop=(k == KD - 1))
                    for h in range(2):
                        src3 = ps[:, h * 512:(h + 1) * 512].rearrange(
                            "p (g c) -> p g c", c=64)
                        bv3 = bvb_sb[:].rearrange("p (g c) -> p g c", c=64)
                        dst3 = Vt[s2 * 2 + h][:, :].rearrange(
                            "p (g c) -> p g c", c=65)[:, :, 0:64]
                        nc.vector.tensor_add(dst3, src3, bv3)

                # ---- attention; normalize deferred by one iteration ----
                pending = None

                def emit_normalize(p):
                    hp, n, us = p
                    sq = slice(n * 512, (n + 1) * 512)
                    for half, u in ((0, us[0]), (1, us[1])):
                        r = rrp.tile([128, 512], bf16, tag="r",
                                     name=f"r{hp}{n}{half}")
                        with nc.allow_low_precision(
                                reason="bf16 softmax denom matches bf16 "
                                       "matmul precision"):
                            nc.vector.reciprocal(r[64:65, :], u[64:65, :])
                        pb = ps_s(f"pb{hp}{n}{half}")
                        nc.tensor.matmul(pb[0:64, 0:512],
                                         lhsT=ones_sb[64:65, 0:64],
                                         rhs=r[64:65, :],
                                         start=True, stop=True)
                        if half == 0:
                            nc.vector.tensor_mul(
                                OT[hp][0:64, sq], u[0:64, :],
                                pb[0:64, 0:512])
                        else:
                            stB = stgp.tile([64, 512], bf16, tag="st",
                                            name=f"stB{hp}{n}")
                            nc.vector.tensor_mul(stB[:], u[0:64, :],
                                                 pb[0:64, 0:512])
                            nc.sync.dma_start(OT[hp][64:128, sq], stB[:])

                for n in range(NS):
                    sq = slice(n * 512, (n + 1) * 512)
                    for hp in range(NQ):
                        oA = psp.tile([128, 512], f32, tag="o", bufs=2,
                                      name=f"oA{hp}{n}")
                        oB = psp.tile([128, 512], f32, tag="o", bufs=2,
                                      name=f"oB{hp}{n}")
                        for j in range(ST):
                            sk = slice(j * 128, (j + 1) * 128)
                            # both heads' scores in one 2-bank tile; the two
                            # K=64 matmuls row-tile and overlap in the PE
                            sS = ps_s(f"sS{hp}{n}{j}")
                            nc.tensor.matmul(
                                sS[:, 0:512], lhsT=KT[hp][0:64, sk],
                                rhs=QT[hp][0:64, sq],
                                start=True, stop=True)
                            nc.tensor.matmul(
                                sS[:, 512:1024], lhsT=KT[hp][64:128, sk],
                                rhs=QT[hp][64:128, sq],
                                start=True, stop=True)
                            pT = pbufp.tile([128, 1024], bf16, tag="p",
                                            name=f"pT{hp}{n}{j}")
                            nc.scalar.activation(pT[:], sS[:], EXP,
                                                 scale=SCALE)
                            ha = hp * 2
                            nc.tensor.matmul(
                                oA[0:65, :],
                                lhsT=Vt[j][:, ha * 65:ha * 65 + 65],
                                rhs=pT[:, 0:512],
                                start=(j == 0), stop=(j == ST - 1))
                            nc.tensor.matmul(
                                oB[0:65, :],
                                lhsT=Vt[j][:, ha * 65 + 65:ha * 65 + 130],
                                rhs=pT[:, 512:1024],
                                start=(j == 0), stop=(j == ST - 1))
                        # evacuate psum accumulators to SBUF right away
                        us = []
                        for half, oPS in ((0, oA), (1, oB)):
                            u = unp.tile([128, 512], f32, tag="u",
                                         name=f"u{hp}{n}{half}")
                            nc.vector.tensor_copy(u[0:65, :], oPS[0:65, :])
                            us.append(u)
                        if pending is not None:
                            emit_normalize(pending)
                        pending = (hp, n, us)
                emit_normalize(pending)

                # ---- output projection (partial over this head-group) ----
                for m in range(ST):
                    ps = ps_s(f"pj{m}")
                    for k in range(NQ):
                        for h in range(2):
                            nc.tensor.matmul(
                                ps[:, h * 512:(h + 1) * 512],
                                lhsT=OT[k][:, m * 128:(m + 1) * 128],
                                rhs=wps[k][:, h * 512:(h + 1) * 512],
                                start=(k == 0), stop=(k == NQ - 1))
                    ob = stgp.tile([128, 1024], f32, tag="ob",
                                   name=f"ob{m}")
                    nc.vector.tensor_copy(ob[:], ps[:])
                    nc.sync.dma_start(out[m * 128:(m + 1) * 128, :], ob[:])
    nc.compile()
    return nc


def _get_nc():
    if "nc" not in _CACHE:
        _CACHE["nc"] = _build_bass()
    return _CACHE["nc"]


def _in_maps(x, w_qkv, b_qkv, w_proj, b_proj):
    x = np.asarray(x, np.float32)
    w_qkv = np.asarray(w_qkv, np.float32)
    b_qkv = np.asarray(b_qkv, np.float32)
    w_proj = np.asarray(w_proj, np.float32)

    def bias_bcast(b512):
        # [128, 4096]: m-tile blocks of 1024 cols, value per partition
        col = b512.reshape(4, 128).T[:, :, None]            # [128, 4, 1]
        return np.ascontiguousarray(
            np.broadcast_to(col, (128, 4, 1024)).reshape(128, 4096))

    maps = []
    for c in range(N_CORES):
        b, g = divmod(c, 2)
        cols = slice(g * GC, (g + 1) * GC)
        wqs = w_qkv[:, 0 * DIM:1 * DIM][:, cols]
        wks = w_qkv[:, 1 * DIM:2 * DIM][:, cols]
        wvs = w_qkv[:, 2 * DIM:3 * DIM][:, cols]
        bqs = b_qkv[0 * DIM:1 * DIM][cols]
        bks = b_qkv[1 * DIM:2 * DIM][cols]
        bvs = b_qkv[2 * DIM:3 * DIM][cols]
        rows = slice(g * GC, (g + 1) * GC)
        maps.append({
            "xT": np.ascontiguousarray(x[b].T).astype(BF),
            "wq": wqs.astype(BF),
            "wk": wks.astype(BF),
            "wv": wvs.astype(BF),
            "wp": w_proj[rows, :].astype(BF),
            "bq": bias_bcast(bqs),
            "bk": bias_bcast(bks),
            "bvb": np.broadcast_to(bvs, (128, GC)).copy(),
        })
    return maps


def kernel(x, w_qkv, b_qkv, w_proj, b_proj, _trace=False):
    import time
    from concourse import bass_utils
    nc = _get_nc()
    maps = _in_maps(x, w_qkv, b_qkv, w_proj, b_proj)
    try:
        res = bass_utils.run_bass_kernel_spmd(nc, maps,
                                              core_ids=list(range(N_CORES)),
                                              trace=_trace)
    except Exception:
        # a previously wedged device usually clears after one failed
        # attempt; retry once
        time.sleep(5)
        res = bass_utils.run_bass_kernel_spmd(nc, maps,
                                              core_ids=list(range(N_CORES)),
                                              trace=_trace)
    _CACHE["last_result"] = res
    b_proj = np.asarray(b_proj, np.float32)
    outs = np.empty((B, S, DIM), np.float32)
    for b in range(B):
        outs[b] = (res.results[2 * b]["out"] + res.results[2 * b + 1]["out"]
                   + b_proj)
    return outs



# revision 9
# speedup vs baseline: 1.0670x; 1.0670x over previous
"""Trainium2 Bass kernel for 16-head attention (B=4, S=2048, D=1024).

Sharding: 8 cores = 4 batches x 2 head-groups. Core c handles batch c//2,
heads (c%2)*8 .. +8. Each core computes a partial projection output
[S, D]; the host sums the two head-group partials per batch and adds
b_proj. No collectives.

The attention inner loop is scalar-engine bound (exp of the score matrix:
256 ACTIVATEs x ~1.1us = ~285us of ACT work per core), so the kernel is
built as two decoupled streams: an S-stream (scores matmul pair + exp)
that runs L=16 steps ahead of an A-stream (attn@V accumulation +
normalize), connected by a deep pT pool. The PE therefore never stalls
on an exp, and the gaps the PE does have inside each ~1.1us exp period
are filled by drip-fed "filler" units: the V/K/Q projection half-chains
early on, the output-projection tiles at the end. Scores are computed
transposed ([sk, sq]) with two heads row-tiled per PE pass; V is
ones-augmented so attn@V also yields softmax row-sums; normalization
uses a DVE reciprocal + a K=1 outer-product matmul, staggered across
A-stream block boundaries.
"""

import sys
import os

sys.path.insert(0, "/opt/trn_rl_repo")

import numpy as np
import ml_dtypes

BF = ml_dtypes.bfloat16

DIM = 1024
N_HEADS = 16
HD = 64
B = 4
S = 2048
HPC = 8          # heads per core
GC = HPC * HD    # 512 columns per head-group
N_CORES = 8
SCALE = HD ** -0.5
LAG = 16         # S-stream leads A-stream by this many steps

_CACHE = {}


def _build_bass():
    import concourse.bass as bass
    import concourse.mybir as mybir
    import concourse.tile as tile
    from concourse import bacc

    f32 = mybir.dt.float32
    bf16 = mybir.dt.bfloat16
    EXP = mybir.ActivationFunctionType.Exp

    nc = bacc.Bacc("TRN2", target_bir_lowering=False, debug=False,
                   num_devices=N_CORES)

    xT = nc.dram_tensor("xT", [DIM, S], bf16, kind="ExternalInput").ap()
    wq = nc.dram_tensor("wq", [DIM, GC], bf16, kind="ExternalInput").ap()
    wk = nc.dram_tensor("wk", [DIM, GC], bf16, kind="ExternalInput").ap()
    wv = nc.dram_tensor("wv", [DIM, GC], bf16, kind="ExternalInput").ap()
    wp = nc.dram_tensor("wp", [GC, DIM], bf16, kind="ExternalInput").ap()
    # per-partition qkv bias columns: bqk[p, m] = b_q[m*128+p] (m<4), b_k (m-4)
    bqk = nc.dram_tensor("bqk", [128, 8], f32, kind="ExternalInput").ap()
    # v bias broadcast across partitions (small)
    bvb = nc.dram_tensor("bvb", [128, GC], f32, kind="ExternalInput").ap()
    out = nc.dram_tensor("out", [S, DIM], f32, kind="ExternalOutput").ap()

    KD = DIM // 128   # 8 k-tiles over D
    NQ = GC // 128    # 4 m-tiles over the 512 head-group columns
    NS = S // 512     # 4 seq chunks of 512
    ST = S // 128     # 16 seq tiles of 128
    NSTEP = NQ * NS * ST  # 256

    with tile.TileContext(nc) as tc:
        with tc.tile_pool(name="const", bufs=1) as cp:
            ones_sb = cp.tile([128, 64], bf16, name="ones_sb")
            nc.any.memset(ones_sb[:], 1.0)
            # warm the ACT exp table during the input DMAs
            dummy = cp.tile([1, 16], bf16, name="dummy")
            nc.scalar.activation(dummy[:], ones_sb[0:1, 0:16], EXP)

            bqk_sb = cp.tile([128, 8], f32, name="bqk_sb")
            nc.scalar.dma_start(bqk_sb[:], bqk[:, :])

            # inputs spread over engine DMA queues: xT on sync, wv/wk on
            # gpsimd, wq/wp on scalar. wk/wq m=0 column slices land first so
            # the first scores chain starts as soon as xT is in.
            xTs, wqs, wks, wvs = [], [], [], []
            for k in range(KD):
                t = cp.tile([128, S], bf16, name=f"xs{k}")
                nc.sync.dma_start(t[:], xT[k * 128:(k + 1) * 128, :])
                xTs.append(t)
                tv = cp.tile([128, GC], bf16, name=f"wvs{k}")
                nc.gpsimd.dma_start(tv[:], wv[k * 128:(k + 1) * 128, :])
                wvs.append(tv)
                tk = cp.tile([128, GC], bf16, name=f"wks{k}")
                nc.gpsimd.dma_start(tk[:, 0:128],
                                    wk[k * 128:(k + 1) * 128, 0:128])
                wks.append(tk)
                tq = cp.tile([128, GC], bf16, name=f"wqs{k}")
                nc.scalar.dma_start(tq[:, 0:128],
                                    wq[k * 128:(k + 1) * 128, 0:128])
                wqs.append(tq)
            bvb_sb = cp.tile([128, GC], f32, name="bvb_sb")
            nc.gpsimd.dma_start(bvb_sb[:], bvb[:, :])
            # non-critical weight slices queue on sync BEHIND xT so they
            # don't steal HBM bandwidth from the startup-critical stream
            for k in range(KD):
                nc.sync.dma_start(wks[k][:, 128:GC],
                                  wk[k * 128:(k + 1) * 128, 128:GC])
                nc.sync.dma_start(wqs[k][:, 128:GC],
                                  wq[k * 128:(k + 1) * 128, 128:GC])
            wps = []
            for k in range(NQ):
                t = cp.tile([128, DIM], bf16, name=f"wps{k}")
                nc.sync.dma_start(t[:], wp[k * 128:(k + 1) * 128, :])
                wps.append(t)

            QT = [cp.tile([128, S], bf16, name=f"QT{m}") for m in range(NQ)]
            KT = [cp.tile([128, S], bf16, name=f"KT{m}") for m in range(NQ)]
            # V tiles: per head 65 cols (64 data + trailing ones column)
            Vt = [cp.tile([128, HPC * 65], bf16, name=f"Vt{s}")
                  for s in range(ST)]
            OT = [cp.tile([128, S], bf16, name=f"OT{m}") for m in range(NQ)]

            for s in range(ST):
                ones_cols = Vt[s][:, :].rearrange(
                    "p (h c) -> p h c", c=65)[:, :, 64:65]
                nc.any.memset(ones_cols, 1.0)

            with tc.tile_pool(name="ps", bufs=1, space="PSUM") as psp, \
                 tc.tile_pool(name="pbuf", bufs=LAG + 2) as pbufp, \
                 tc.tile_pool(name="un", bufs=4) as unp, \
                 tc.tile_pool(name="rr", bufs=2) as rrp, \
                 tc.tile_pool(name="stg", bufs=3) as stgp:

                def ps_s(name):          # scores: 2 banks x2
                    return psp.tile([128, 1024], f32, tag="s", bufs=2,
                                    name=name)

                def ps_f(name):          # qkv/proj half-chains, pb: 1 bank x2
                    return psp.tile([128, 512], f32, tag="f", bufs=2,
                                    name=name)

                # ---- V half-chain units: Vt[s] per unit pair --------------
                def v_units(s):
                    state = {}

                    def first():
                        ps = ps_f(f"v{s}")
                        state["ps"] = ps
                        for k in range(4):
                            nc.tensor.matmul(
                                ps[:], lhsT=xTs[k][:, s * 128:(s + 1) * 128],
                                rhs=wvs[k][:, :],
                                start=(k == 0), stop=False)

                    def second():
                        ps = state["ps"]
                        for k in range(4, KD):
                            nc.tensor.matmul(
                                ps[:], lhsT=xTs[k][:, s * 128:(s + 1) * 128],
                                rhs=wvs[k][:, :],
                                start=False, stop=(k == KD - 1))
                        src3 = ps[:].rearrange("p (g c) -> p g c", c=64)
                        bv3 = bvb_sb[:].rearrange("p (g c) -> p g c", c=64)
                        dst3 = Vt[s][:, :].rearrange(
                            "p (h c) -> p h c", c=65)[:, :, 0:64]
                        nc.vector.tensor_add(dst3, src3, bv3)

                    yield first
                    yield second

                # ---- Q/K half-chain units ---------------------------------
                def qk_units(is_k, m, n2, h):
                    ws = wks if is_k else wqs
                    dst = KT[m] if is_k else QT[m]
                    bcol = bqk_sb[:, 4 + m:5 + m] if is_k else \
                        bqk_sb[:, m:m + 1]
                    state = {}

                    def first():
                        ps = ps_f(f"{'k' if is_k else 'q'}{m}{n2}{h}")
                        state["ps"] = ps
                        for k in range(4):
                            nc.tensor.matmul(
                                ps[:],
                                lhsT=ws[k][:, m * 128:(m + 1) * 128],
                                rhs=xTs[k][:, (n2 * 2 + h) * 512:
                                           (n2 * 2 + h + 1) * 512],
                                start=(k == 0), stop=False)

                    def second():
                        ps = state["ps"]
                        for k in range(4, KD):
                            nc.tensor.matmul(
                                ps[:],
                                lhsT=ws[k][:, m * 128:(m + 1) * 128],
                                rhs=xTs[k][:, (n2 * 2 + h) * 512:
                                           (n2 * 2 + h + 1) * 512],
                                start=False, stop=(k == KD - 1))
                        nc.vector.tensor_scalar_add(
                            dst[:, (n2 * 2 + h) * 512:(n2 * 2 + h + 1) * 512],
                            ps[:], bcol)

                    yield first
                    yield second

                def emit_qk_half(is_k, m, n2, h):
                    for u in qk_units(is_k, m, n2, h):
                        u()

                # ---- proj half units --------------------------------------
                def proj_units(mt, h):
                    def go():
                        ps = ps_f(f"pj{mt}{h}")
                        for k in range(NQ):
                            nc.tensor.matmul(
                                ps[:],
                                lhsT=OT[k][:, mt * 128:(mt + 1) * 128],
                                rhs=wps[k][:, h * 512:(h + 1) * 512],
                                start=(k == 0), stop=(k == NQ - 1))
                        ob = stgp.tile([128, 512], f32, tag="ob",
                                       name=f"ob{mt}{h}")
                        nc.vector.tensor_copy(ob[:], ps[:])
                        nc.sync.dma_start(
                            out[mt * 128:(mt + 1) * 128,
                                h * 512:(h + 1) * 512], ob[:])
                    yield go

                # ---- startup: just enough for the first scores ------------
                emit_qk_half(True, 0, 0, 0)    # KT[0] sk 0:512  (j=0..3)
                emit_qk_half(False, 0, 0, 0)   # QT[0] sq 0:512

                # ---- filler generator (ordered by first-need tick) --------
                def gen_fillers():
                    yield from qk_units(True, 0, 0, 1)    # j=4..7
                    yield from v_units(0)
                    yield from v_units(1)
                    yield from qk_units(True, 0, 1, 0)    # j=8..11
                    yield from v_units(2)
                    yield from v_units(3)
                    yield from qk_units(True, 0, 1, 1)    # j=12..15
                    yield from v_units(4)
                    yield from v_units(5)
                    yield from qk_units(False, 0, 0, 1)   # sq 512:1024 @t16
                    yield from v_units(6)
                    yield from v_units(7)
                    yield from v_units(8)
                    yield from qk_units(False, 0, 1, 0)   # sq 1024:1536 @t32
                    yield from v_units(9)
                    yield from v_units(10)
                    yield from v_units(11)
                    yield from qk_units(False, 0, 1, 1)   # sq 1536:2048 @t48
                    yield from v_units(12)
                    yield from v_units(13)
                    yield from v_units(14)
                    yield from v_units(15)
                    for m in range(1, NQ):
                        for n2, h, is_k in ((0, 0, True), (0, 1, True),
                                            (0, 0, False), (0, 1, False),
                                            (1, 0, True), (1, 1, True),
                                            (1, 0, False), (1, 1, False)):
                            yield from qk_units(is_k, m, n2, h)

                fillers = gen_fillers()
                proj_queue = []

                def pull(n_units):
                    for _ in range(n_units):
                        u = next(fillers, None)
                        if u is not None:
                            u()
                        elif proj_queue:
                            proj_queue.pop(0)()

                # ---- normalize (staggered halves) -------------------------
                # the [1,512] softmax-sum row is DMA-respread to [128,4] so
                # the DVE reciprocal runs at 128 lanes (~165ns, not 3.3us)
                def normalize_dve(p):
                    hp, n, us, st = p
                    for half, u in ((0, us[0]), (1, us[1])):
                        r4 = rrp.tile([128, 4], f32, tag="r4",
                                      name=f"r4{hp}{n}{half}")
                        nc.gpsimd.dma_start(r4[:], u[64:65, :])
                        rf4 = rrp.tile([128, 4], f32, tag="rf4",
                                       name=f"rf4{hp}{n}{half}")
                        nc.vector.reciprocal(rf4[:], r4[:])
                        rb4 = rrp.tile([128, 4], bf16, tag="rb4",
                                       name=f"rb4{hp}{n}{half}")
                        with nc.allow_low_precision(
                                reason="bf16 softmax denom matches bf16 "
                                       "matmul precision"):
                            nc.vector.tensor_copy(rb4[:], rf4[:])
                        rb = rrp.tile([128, 512], bf16, tag="rb",
                                      name=f"rb{hp}{n}{half}")
                        nc.gpsimd.dma_start(rb[64:65, :], rb4[:])
                        st.append(rb)

                def normalize_pe(p):
                    hp, n, us, st = p
                    sq = slice(n * 512, (n + 1) * 512)
                    for half, u in ((0, us[0]), (1, us[1])):
                        rb = st[half]
                        pb = ps_f(f"pb{hp}{n}{half}")
                        nc.tensor.matmul(pb[0:64, 0:512],
                                         lhsT=ones_sb[64:65, 0:64],
                                         rhs=rb[64:65, :],
                                         start=True, stop=True)
                        if half == 0:
                            nc.vector.tensor_mul(
                                OT[hp][0:64, sq], u[0:64, :],
                                pb[0:64, 0:512])
                        else:
                            stB = stgp.tile([64, 512], bf16, tag="st",
                                            name=f"stB{hp}{n}")
                            nc.vector.tensor_mul(stB[:], u[0:64, :],
                                                 pb[0:64, 0:512])
                            nc.sync.dma_start(OT[hp][64:128, sq], stB[:])

                # ---- main loop: S-stream leads A-stream by LAG ------------
                pending = None
                pTs = {}
                ablk = {}

                for g in range(NSTEP + LAG):
                    if g < NSTEP:
                        hp, n, j = g // 64, (g // 16) % 4, g % 16
                        sq = slice(n * 512, (n + 1) * 512)
                        sk = slice(j * 128, (j + 1) * 128)
                        sS = ps_s(f"sS{g}")
                        nc.tensor.matmul(
                            sS[:, 0:512], lhsT=KT[hp][0:64, sk],
                            rhs=QT[hp][0:64, sq],
                            start=True, stop=True)
                        nc.tensor.matmul(
                            sS[:, 512:1024], lhsT=KT[hp][64:128, sk],
                            rhs=QT[hp][64:128, sq],
                            start=True, stop=True)
                        pT = pbufp.tile([128, 1024], bf16, tag="p",
                                        name=f"pT{g}")
                        nc.scalar.activation(pT[:], sS[:], EXP, scale=SCALE)
                        pTs[g] = pT
                    a = g - LAG
                    if a >= 0:
                        hp, n, j = a // 64, (a // 16) % 4, a % 16
                        if j == 0:
                            if pending is not None:
                                normalize_dve(pending)
                            ablk["oA"] = psp.tile([128, 512], f32, tag="o",
                                                  bufs=2, name=f"oA{a}")
                            ablk["oB"] = psp.tile([128, 512], f32, tag="o",
                                                  bufs=2, name=f"oB{a}")
                        pT = pTs.pop(a)
                        ha = hp * 2
                        nc.tensor.matmul(
                            ablk["oA"][0:65, :],
                            lhsT=Vt[j][:, ha * 65:ha * 65 + 65],
                            rhs=pT[:, 0:512],
                            start=(j == 0), stop=(j == ST - 1))
                        nc.tensor.matmul(
                            ablk["oB"][0:65, :],
                            lhsT=Vt[j][:, ha * 65 + 65:ha * 65 + 130],
                            rhs=pT[:, 512:1024],
                            start=(j == 0), stop=(j == ST - 1))
                        if j == 6 and pending is not None:
                            php, pn = pending[0], pending[1]
                            normalize_pe(pending)
                            pending = None
                            if php == NQ - 1:
                                # OT chunk pn now fully written: release proj
                                for mt in range(4 * pn, 4 * pn + 4):
                                    for h in range(2):
                                        proj_queue.extend(proj_units(mt, h))
                        if j == ST - 1:
                            us = []
                            for half, oPS in ((0, ablk["oA"]),
                                              (1, ablk["oB"])):
                                u = unp.tile([128, 512], f32, tag="u",
                                             name=f"u{a}{half}")
                                nc.vector.tensor_copy(u[0:65, :],
                                                      oPS[0:65, :])
                                us.append(u)
                            pending = (hp, n, us, [])
                    # filler pacing: 2/tick through the DMA-paced warmup,
                    # taper to 1 per 2 ticks in steady state
                    if g < 16:
                        pull(2)
                    elif g < 32:
                        pull(2 if g % 2 == 0 else 1)
                    elif g % 2 == 0:
                        pull(1)

                # tail: last block's normalize + remaining proj
                normalize_dve(pending)
                normalize_pe(pending)
                for mt in range(12, 16):
                    for h in range(2):
                        proj_queue.extend(proj_units(mt, h))
                while proj_queue:
                    proj_queue.pop(0)()
    nc.compile()
    return nc


def _get_nc():
    if "nc" not in _CACHE:
        _CACHE["nc"] = _build_bass()
    return _CACHE["nc"]


def _in_maps(x, w_qkv, b_qkv, w_proj, b_proj):
    x = np.asarray(x, np.float32)
    w_qkv = np.asarray(w_qkv, np.float32)
    b_qkv = np.asarray(b_qkv, np.float32)
    w_proj = np.asarray(w_proj, np.float32)

    maps = []
    for c in range(N_CORES):
        b, g = divmod(c, 2)
        cols = slice(g * GC, (g + 1) * GC)
        wqs = w_qkv[:, 0 * DIM:1 * DIM][:, cols]
        wks = w_qkv[:, 1 * DIM:2 * DIM][:, cols]
        wvs = w_qkv[:, 2 * DIM:3 * DIM][:, cols]
        bqs = b_qkv[0 * DIM:1 * DIM][cols]
        bks = b_qkv[1 * DIM:2 * DIM][cols]
        bvs = b_qkv[2 * DIM:3 * DIM][cols]
        rows = slice(g * GC, (g + 1) * GC)
        bqk = np.concatenate([bqs.reshape(4, 128).T,
                              bks.reshape(4, 128).T], axis=1)
        maps.append({
            "xT": np.ascontiguousarray(x[b].T).astype(BF),
            "wq": wqs.astype(BF),
            "wk": wks.astype(BF),
            "wv": wvs.astype(BF),
            "wp": w_proj[rows, :].astype(BF),
            "bqk": np.ascontiguousarray(bqk, dtype=np.float32),
            "bvb": np.broadcast_to(bvs, (128, GC)).copy(),
        })
    return maps


def kernel(x, w_qkv, b_qkv, w_proj, b_proj, _trace=False):
    import time
    from concourse import bass_utils
    nc = _get_nc()
    maps = _in_maps(x, w_qkv, b_qkv, w_proj, b_proj)
    try:
        res = bass_utils.run_bass_kernel_spmd(nc, maps,
                                              core_ids=list(range(N_CORES)),
                                              trace=_trace)
    except Exception:
        # a previously wedged device usually clears after one failed
        # attempt; retry once
        time.sleep(5)
        res = bass_utils.run_bass_kernel_spmd(nc, maps,
                                              core_ids=list(range(N_CORES)),
                                              trace=_trace)
    _CACHE["last_result"] = res
    b_proj = np.asarray(b_proj, np.float32)
    outs = np.empty((B, S, DIM), np.float32)
    for b in range(B):
        outs[b] = (res.results[2 * b]["out"] + res.results[2 * b + 1]["out"]
                   + b_proj)
    return outs


# revision 15
# speedup vs baseline: 1.0910x; 1.0225x over previous
"""Trainium2 Bass kernel for 16-head attention (B=4, S=2048, D=1024).

Sharding: 8 cores = 4 batches x 2 head-groups. Core c handles batch c//2,
heads (c%2)*8 .. +8. Each core computes a partial projection output
[S, D]; the host sums the two head-group partials per batch and adds
b_proj. No collectives.

The attention inner loop is scalar-engine bound (exp of the score matrix:
256 ACTIVATEs x ~1.1us = ~285us of ACT work per core), so the kernel is
built as two decoupled streams: an S-stream (scores matmul pair + exp)
that runs L=16 steps ahead of an A-stream (attn@V accumulation +
normalize), connected by a deep pT pool. The PE therefore never stalls
on an exp, and the gaps the PE does have inside each ~1.1us exp period
are filled by drip-fed "filler" units: the V/K/Q projection half-chains
early on, the output-projection tiles at the end. Scores are computed
transposed ([sk, sq]) with two heads row-tiled per PE pass; V is
ones-augmented so attn@V also yields softmax row-sums; normalization
uses a DVE reciprocal + a K=1 outer-product matmul, staggered across
A-stream block boundaries.
"""

import sys
import os

sys.path.insert(0, "/opt/trn_rl_repo")

import numpy as np
import ml_dtypes

BF = ml_dtypes.bfloat16

DIM = 1024
N_HEADS = 16
HD = 64
B = 4
S = 2048
HPC = 8          # heads per core
GC = HPC * HD    # 512 columns per head-group
N_CORES = 8
SCALE = HD ** -0.5
LAG = 16         # S-stream leads A-stream by this many steps

_CACHE = {}


def _build_bass():
    import concourse.bass as bass
    import concourse.mybir as mybir
    import concourse.tile as tile
    from concourse import bacc

    f32 = mybir.dt.float32
    bf16 = mybir.dt.bfloat16
    EXP = mybir.ActivationFunctionType.Exp

    nc = bacc.Bacc("TRN2", target_bir_lowering=False, debug=False,
                   num_devices=N_CORES)

    xT = nc.dram_tensor("xT", [DIM, S], bf16, kind="ExternalInput").ap()
    wq = nc.dram_tensor("wq", [DIM, GC], bf16, kind="ExternalInput").ap()
    wk = nc.dram_tensor("wk", [DIM, GC], bf16, kind="ExternalInput").ap()
    wv = nc.dram_tensor("wv", [DIM, GC], bf16, kind="ExternalInput").ap()
    wp = nc.dram_tensor("wp", [GC, DIM], bf16, kind="ExternalInput").ap()
    # per-partition qkv bias columns: bqk[p, m] = b_q[m*128+p] (m<4), b_k (m-4)
    bqk = nc.dram_tensor("bqk", [128, 8], f32, kind="ExternalInput").ap()
    # v bias broadcast across partitions (small)
    bvb = nc.dram_tensor("bvb", [128, GC], f32, kind="ExternalInput").ap()
    out = nc.dram_tensor("out", [S, DIM], f32, kind="ExternalOutput").ap()

    KD = DIM // 128   # 8 k-tiles over D
    NQ = GC // 128    # 4 m-tiles over the 512 head-group columns
    NS = S // 512     # 4 seq chunks of 512
    ST = S // 128     # 16 seq tiles of 128
    NSTEP = NQ * NS * ST  # 256

    with tile.TileContext(nc) as tc:
        with tc.tile_pool(name="const", bufs=1) as cp:
            ones_sb = cp.tile([128, 64], bf16, name="ones_sb")
            nc.any.memset(ones_sb[:], 1.0)
            # warm the ACT exp table during the input DMAs
            dummy = cp.tile([1, 16], bf16, name="dummy")
            nc.scalar.activation(dummy[:], ones_sb[0:1, 0:16], EXP)

            bqk_sb = cp.tile([128, 8], f32, name="bqk_sb")
            nc.scalar.dma_start(bqk_sb[:], bqk[:, :])

            # inputs spread over engine DMA queues: xT on sync, wv/wk on
            # gpsimd, wq/wp on scalar. wk/wq m=0 column slices land first so
            # the first scores chain starts as soon as xT is in.
            xTs, wqs, wks, wvs = [], [], [], []
            for k in range(KD):
                t = cp.tile([128, S], bf16, name=f"xs{k}")
                nc.sync.dma_start(t[:], xT[k * 128:(k + 1) * 128, :])
                xTs.append(t)
                tv = cp.tile([128, GC], bf16, name=f"wvs{k}")
                wvs.append(tv)
                tk = cp.tile([128, GC], bf16, name=f"wks{k}")
                nc.gpsimd.dma_start(tk[:, 0:128],
                                    wk[k * 128:(k + 1) * 128, 0:128])
                wks.append(tk)
                tq = cp.tile([128, GC], bf16, name=f"wqs{k}")
                nc.scalar.dma_start(tq[:, 0:128],
                                    wq[k * 128:(k + 1) * 128, 0:128])
                wqs.append(tq)
            # wv and the non-critical weight slices queue on sync BEHIND xT
            # so the startup-critical stream (xT + wk/wq m0 slices) gets the
            # full HBM bandwidth; V matmuls need all of xT first anyway
            for k in range(KD):
                nc.sync.dma_start(wvs[k][:], wv[k * 128:(k + 1) * 128, :])
            bvb_sb = cp.tile([128, GC], f32, name="bvb_sb")
            nc.gpsimd.dma_start(bvb_sb[:], bvb[:, :])
            for k in range(KD):
                nc.sync.dma_start(wks[k][:, 128:GC],
                                  wk[k * 128:(k + 1) * 128, 128:GC])
                nc.sync.dma_start(wqs[k][:, 128:GC],
                                  wq[k * 128:(k + 1) * 128, 128:GC])
            wps = []
            for k in range(NQ):
                t = cp.tile([128, DIM], bf16, name=f"wps{k}")
                nc.sync.dma_start(t[:], wp[k * 128:(k + 1) * 128, :])
                wps.append(t)

            QT = [cp.tile([128, S], bf16, name=f"QT{m}") for m in range(NQ)]
            KT = [cp.tile([128, S], bf16, name=f"KT{m}") for m in range(NQ)]
            # V tiles: per head 65 cols (64 data + trailing ones column)
            Vt = [cp.tile([128, HPC * 65], bf16, name=f"Vt{s}")
                  for s in range(ST)]
            OT = [cp.tile([128, S], bf16, name=f"OT{m}") for m in range(NQ)]

            for s in range(ST):
                ones_cols = Vt[s][:, :].rearrange(
                    "p (h c) -> p h c", c=65)[:, :, 64:65]
                nc.any.memset(ones_cols, 1.0)

            with tc.tile_pool(name="ps", bufs=1, space="PSUM") as psp, \
                 tc.tile_pool(name="pbuf", bufs=LAG + 2) as pbufp, \
                 tc.tile_pool(name="un", bufs=4) as unp, \
                 tc.tile_pool(name="rr", bufs=2) as rrp, \
                 tc.tile_pool(name="stg", bufs=3) as stgp:

                def ps_s(name):          # scores: 2 banks x2
                    return psp.tile([128, 1024], f32, tag="s", bufs=2,
                                    name=name)

                def ps_f(name):          # qkv/proj half-chains, pb: 1 bank x2
                    return psp.tile([128, 512], f32, tag="f", bufs=2,
                                    name=name)

                # ---- V half-chain units: Vt[s] per 4-unit chain -----------
                def v_units(s):
                    state = {}

                    def part(k0):
                        def go():
                            if k0 == 0:
                                state["ps"] = ps_f(f"v{s}")
                            ps = state["ps"]
                            for k in range(k0, k0 + 2):
                                nc.tensor.matmul(
                                    ps[:],
                                    lhsT=xTs[k][:, s * 128:(s + 1) * 128],
                                    rhs=wvs[k][:, :],
                                    start=(k == 0), stop=(k == KD - 1))
                            if k0 == KD - 2:
                                src3 = ps[:].rearrange(
                                    "p (g c) -> p g c", c=64)
                                bv3 = bvb_sb[:].rearrange(
                                    "p (g c) -> p g c", c=64)
                                dst3 = Vt[s][:, :].rearrange(
                                    "p (h c) -> p h c", c=65)[:, :, 0:64]
                                nc.vector.tensor_add(dst3, src3, bv3)
                        return go

                    for k0 in range(0, KD, 2):
                        yield part(k0)

                # ---- Q/K half-chain units (4 x 2-matmul units) ------------
                def qk_units(is_k, m, n2, h):
                    ws = wks if is_k else wqs
                    dst = KT[m] if is_k else QT[m]
                    bcol = bqk_sb[:, 4 + m:5 + m] if is_k else \
                        bqk_sb[:, m:m + 1]
                    state = {}

                    def part(k0):
                        def go():
                            if k0 == 0:
                                state["ps"] = ps_f(
                                    f"{'k' if is_k else 'q'}{m}{n2}{h}")
                            ps = state["ps"]
                            for k in range(k0, k0 + 2):
                                nc.tensor.matmul(
                                    ps[:],
                                    lhsT=ws[k][:, m * 128:(m + 1) * 128],
                                    rhs=xTs[k][:, (n2 * 2 + h) * 512:
                                               (n2 * 2 + h + 1) * 512],
                                    start=(k == 0), stop=(k == KD - 1))
                            if k0 == KD - 2:
                                nc.vector.tensor_scalar_add(
                                    dst[:, (n2 * 2 + h) * 512:
                                        (n2 * 2 + h + 1) * 512],
                                    ps[:], bcol)
                        return go

                    for k0 in range(0, KD, 2):
                        yield part(k0)

                def emit_qk_half(is_k, m, n2, h):
                    for u in qk_units(is_k, m, n2, h):
                        u()

                # ---- proj half units (2 x 2-matmul units) -----------------
                def proj_units(mt, h):
                    state = {}

                    def part(k0):
                        def go():
                            if k0 == 0:
                                state["ps"] = ps_f(f"pj{mt}{h}")
                            ps = state["ps"]
                            for k in range(k0, k0 + 2):
                                nc.tensor.matmul(
                                    ps[:],
                                    lhsT=OT[k][:, mt * 128:(mt + 1) * 128],
                                    rhs=wps[k][:, h * 512:(h + 1) * 512],
                                    start=(k == 0), stop=(k == NQ - 1))
                            if k0 == NQ - 2:
                                ob = stgp.tile([128, 512], f32, tag="ob",
                                               name=f"ob{mt}{h}")
                                nc.vector.tensor_copy(ob[:], ps[:])
                                nc.sync.dma_start(
                                    out[mt * 128:(mt + 1) * 128,
                                        h * 512:(h + 1) * 512], ob[:])
                        return go

                    for k0 in range(0, NQ, 2):
                        yield part(k0)

                # ---- startup: just enough for the first scores ------------
                emit_qk_half(True, 0, 0, 0)    # KT[0] sk 0:512  (j=0..3)
                emit_qk_half(False, 0, 0, 0)   # QT[0] sq 0:512

                # ---- filler generator (ordered by first-need tick) --------
                def gen_fillers():
                    yield from qk_units(True, 0, 0, 1)    # j=4..7
                    yield from v_units(0)
                    yield from v_units(1)
                    yield from qk_units(True, 0, 1, 0)    # j=8..11
                    yield from v_units(2)
                    yield from v_units(3)
                    yield from qk_units(True, 0, 1, 1)    # j=12..15
                    yield from v_units(4)
                    yield from v_units(5)
                    yield from qk_units(False, 0, 0, 1)   # sq 512:1024 @t16
                    yield from v_units(6)
                    yield from v_units(7)
                    yield from v_units(8)
                    yield from qk_units(False, 0, 1, 0)   # sq 1024:1536 @t32
                    yield from v_units(9)
                    yield from v_units(10)
                    yield from v_units(11)
                    yield from qk_units(False, 0, 1, 1)   # sq 1536:2048 @t48
                    yield from v_units(12)
                    yield from v_units(13)
                    yield from v_units(14)
                    yield from v_units(15)
                    for m in range(1, NQ):
                        for n2, h, is_k in ((0, 0, True), (0, 1, True),
                                            (0, 0, False), (0, 1, False),
                                            (1, 0, True), (1, 1, True),
                                            (1, 0, False), (1, 1, False)):
                            yield from qk_units(is_k, m, n2, h)

                fillers = gen_fillers()
                proj_queue = []

                def pull(n_units):
                    for _ in range(n_units):
                        u = next(fillers, None)
                        if u is not None:
                            u()
                        elif proj_queue:
                            proj_queue.pop(0)()

                # ---- normalize (staggered halves) -------------------------
                # the [1,512] softmax-sum row is DMA-respread to [128,4] so
                # the DVE reciprocal runs at 128 lanes (~165ns, not 3.3us)
                def normalize_dve(p):
                    hp, n, us, st = p
                    for half, u in ((0, us[0]), (1, us[1])):
                        r4 = rrp.tile([128, 4], f32, tag="r4",
                                      name=f"r4{hp}{n}{half}")
                        nc.gpsimd.dma_start(r4[:], u[64:65, :])
                        rf4 = rrp.tile([128, 4], f32, tag="rf4",
                                       name=f"rf4{hp}{n}{half}")
                        nc.vector.reciprocal(rf4[:], r4[:])
                        rb4 = rrp.tile([128, 4], bf16, tag="rb4",
                                       name=f"rb4{hp}{n}{half}")
                        with nc.allow_low_precision(
                                reason="bf16 softmax denom matches bf16 "
                                       "matmul precision"):
                            nc.vector.tensor_copy(rb4[:], rf4[:])
                        rb = rrp.tile([128, 512], bf16, tag="rb",
                                      name=f"rb{hp}{n}{half}")
                        nc.gpsimd.dma_start(rb[64:65, :], rb4[:])
                        st.append(rb)

                def normalize_pe(p):
                    hp, n, us, st = p
                    sq = slice(n * 512, (n + 1) * 512)
                    for half, u in ((0, us[0]), (1, us[1])):
                        rb = st[half]
                        pb = ps_f(f"pb{hp}{n}{half}")
                        nc.tensor.matmul(pb[0:64, 0:512],
                                         lhsT=ones_sb[64:65, 0:64],
                                         rhs=rb[64:65, :],
                                         start=True, stop=True)
                        if half == 0:
                            nc.vector.tensor_mul(
                                OT[hp][0:64, sq], u[0:64, :],
                                pb[0:64, 0:512])
                        else:
                            stB = stgp.tile([64, 512], bf16, tag="st",
                                            name=f"stB{hp}{n}")
                            nc.vector.tensor_mul(stB[:], u[0:64, :],
                                                 pb[0:64, 0:512])
                            nc.sync.dma_start(OT[hp][64:128, sq], stB[:])

                # ---- main loop: S-stream leads A-stream by LAG ------------
                pending = None
                pTs = {}
                ablk = {}

                for g in range(NSTEP + LAG):
                    if g < NSTEP:
                        hp, n, j = g // 64, (g // 16) % 4, g % 16
                        sq = slice(n * 512, (n + 1) * 512)
                        sk = slice(j * 128, (j + 1) * 128)
                        sS = ps_s(f"sS{g}")
                        nc.tensor.matmul(
                            sS[:, 0:512], lhsT=KT[hp][0:64, sk],
                            rhs=QT[hp][0:64, sq],
                            start=True, stop=True)
                        nc.tensor.matmul(
                            sS[:, 512:1024], lhsT=KT[hp][64:128, sk],
                            rhs=QT[hp][64:128, sq],
                            start=True, stop=True)
                        pT = pbufp.tile([128, 1024], bf16, tag="p",
                                        name=f"pT{g}")
                        nc.scalar.activation(pT[:], sS[:], EXP, scale=SCALE)
                        pTs[g] = pT
                    a = g - LAG
                    if a >= 0:
                        hp, n, j = a // 64, (a // 16) % 4, a % 16
                        if j == 0:
                            if pending is not None:
                                normalize_dve(pending)
                            ablk["oA"] = psp.tile([128, 512], f32, tag="o",
                                                  bufs=2, name=f"oA{a}")
                            ablk["oB"] = psp.tile([128, 512], f32, tag="o",
                                                  bufs=2, name=f"oB{a}")
                        pT = pTs.pop(a)
                        ha = hp * 2
                        nc.tensor.matmul(
                            ablk["oA"][0:65, :],
                            lhsT=Vt[j][:, ha * 65:ha * 65 + 65],
                            rhs=pT[:, 0:512],
                            start=(j == 0), stop=(j == ST - 1))
                        nc.tensor.matmul(
                            ablk["oB"][0:65, :],
                            lhsT=Vt[j][:, ha * 65 + 65:ha * 65 + 130],
                            rhs=pT[:, 512:1024],
                            start=(j == 0), stop=(j == ST - 1))
                        if j == 6 and pending is not None:
                            php, pn = pending[0], pending[1]
                            normalize_pe(pending)
                            pending = None
                            if php == NQ - 1:
                                # OT chunk pn now fully written: release proj
                                for mt in range(4 * pn, 4 * pn + 4):
                                    for h in range(2):
                                        proj_queue.extend(proj_units(mt, h))
                        if j == ST - 1:
                            us = []
                            for half, oPS in ((0, ablk["oA"]),
                                              (1, ablk["oB"])):
                                u = unp.tile([128, 512], f32, tag="u",
                                             name=f"u{a}{half}")
                                nc.vector.tensor_copy(u[0:65, :],
                                                      oPS[0:65, :])
                                us.append(u)
                            pending = (hp, n, us, [])
                    # filler pacing: 3 units/tick through the DMA-paced
                    # warmup (V + remaining K0/Q0), then 1/tick steady
                    pull(3 if g < 32 else 1)

                # tail: last block's normalize + remaining proj
                normalize_dve(pending)
                normalize_pe(pending)
                for mt in range(12, 16):
                    for h in range(2):
                        proj_queue.extend(proj_units(mt, h))
                while proj_queue:
                    proj_queue.pop(0)()
    nc.compile()
    return nc


def _get_nc():
    if "nc" not in _CACHE:
        _CACHE["nc"] = _build_bass()
    return _CACHE["nc"]


def _in_maps(x, w_qkv, b_qkv, w_proj, b_proj):
    x = np.asarray(x, np.float32)
    w_qkv = np.asarray(w_qkv, np.float32)
    b_qkv = np.asarray(b_qkv, np.float32)
    w_proj = np.asarray(w_proj, np.float32)

    maps = []
    for c in range(N_CORES):
        b, g = divmod(c, 2)
        cols = slice(g * GC, (g + 1) * GC)
        wqs = w_qkv[:, 0 * DIM:1 * DIM][:, cols]
        wks = w_qkv[:, 1 * DIM:2 * DIM][:, cols]
        wvs = w_qkv[:, 2 * DIM:3 * DIM][:, cols]
        bqs = b_qkv[0 * DIM:1 * DIM][cols]
        bks = b_qkv[1 * DIM:2 * DIM][cols]
        bvs = b_qkv[2 * DIM:3 * DIM][cols]
        rows = slice(g * GC, (g + 1) * GC)
        bqk = np.concatenate([bqs.reshape(4, 128).T,
                              bks.reshape(4, 128).T], axis=1)
        maps.append({
            "xT": np.ascontiguousarray(x[b].T).astype(BF),
            "wq": wqs.astype(BF),
            "wk": wks.astype(BF),
            "wv": wvs.astype(BF),
            "wp": w_proj[rows, :].astype(BF),
            "bqk": np.ascontiguousarray(bqk, dtype=np.float32),
            "bvb": np.broadcast_to(bvs, (128, GC)).copy(),
        })
    return maps


def kernel(x, w_qkv, b_qkv, w_proj, b_proj, _trace=False):
    import time
    from concourse import bass_utils
    nc = _get_nc()
    maps = _in_maps(x, w_qkv, b_qkv, w_proj, b_proj)
    try:
        res = bass_utils.run_bass_kernel_spmd(nc, maps,
                                              core_ids=list(range(N_CORES)),
                                              trace=_trace)
    except Exception:
        # a previously wedged device usually clears after one failed
        # attempt; retry once
        time.sleep(5)
        res = bass_utils.run_bass_kernel_spmd(nc, maps,
                                              core_ids=list(range(N_CORES)),
                                              trace=_trace)
    _CACHE["last_result"] = res
    b_proj = np.asarray(b_proj, np.float32)
    outs = np.empty((B, S, DIM), np.float32)
    for b in range(B):
        outs[b] = (res.results[2 * b]["out"] + res.results[2 * b + 1]["out"]
                   + b_proj)
    return outs
